# revision 27
# baseline (speedup 1.0000x reference)
"""FARGAN subframe step on 8 Trainium2 NeuronCores.

Strategy: pure data-parallel over batch (8192 rows/core). Feature-major
compute: activations live as bf16 [feature<=128, 512] SBUF chunks; every
matmul is out[Mchunk, 512] += lhsT.T @ rhs with lhsT = host-packed bf16
weight blocks (zero rows for features a chunk carries that a given matmul
doesn't consume). Batch-major <-> feature-major layout changes ride the
DMA transpose xbar (bf16). The pitch-lag gather uses indirect DMA with one
descriptor per row. Elementwise work is split across DVE / ACT / GPSIMD.
"""
import os
import numpy as np
import ml_dtypes

import concourse.bass as bass
from concourse import bacc
import concourse.tile as tile
import concourse.mybir as mybir
from concourse.masks import make_identity
from contextlib import ExitStack

F32 = mybir.dt.float32
BF16 = mybir.dt.bfloat16
I32 = mybir.dt.int32
AF = mybir.ActivationFunctionType
ALU = mybir.AluOpType
AX = mybir.AxisListType

P = 128
NB = 512               # batch tile (psum bank width in fp32)
RG = 4                 # row groups per batch tile
N_CORES = 8
R = 65536 // N_CORES   # rows per core
NBT = R // NB          # batch tiles per core
EXC_PAD = 64
EXCN = EXC_PAD + R * 256 + EXC_PAD

# ---------------------------------------------------------------------------
# feature-major chunk layout: chunk -> [(src, lo, hi, plo)]
# ---------------------------------------------------------------------------
CHUNKS = {
    "C0":  [("s4", 0, 128, 0)],
    "C1":  [("cond", 0, 80, 0), ("s4", 128, 164, 80)],
    "C2":  [("fpitch", 0, 40, 0), ("pred", 0, 44, 40), ("prev", 0, 40, 84)],
    "S1a": [("s1", 0, 128, 0)],
    "S1b": [("s1", 128, 160, 0)],
    "S2":  [("s2", 0, 128, 0)],
    "S3":  [("s3", 0, 128, 0)],
    "T0":  [("t", 0, 128, 0)],
    "T1":  [("t", 128, 192, 0), ("pgf2", 0, 40, 64)],
    "F0":  [("fwc0", 0, 128, 0)],
    "F1":  [("fwc0", 128, 192, 0), ("pgf0", 0, 40, 64)],
    "G1a": [("g1", 0, 128, 0)],
    "G1b": [("g1", 128, 160, 0)],
    "O1a": [("o1", 0, 128, 0)],
    "O1b": [("o1", 128, 160, 0), ("pgf1", 0, 40, 64)],
    "G2":  [("g2", 0, 128, 0)],
    "O2":  [("o2", 0, 128, 0)],
    "G3":  [("g3", 0, 128, 0)],
    "O3":  [("o3", 0, 128, 0)],
    "K0":  [("kt", 0, 128, 0)],
    "V0":  [("v", 0, 128, 0)],
    "P3":  [("pgf3", 0, 40, 0)],
}

# matmul table: name -> (weight key, inmap {src: dcol}, [(kchunk, K)], [mblock rows])
# mblock rows: list of (lo, hi) row ranges of the effective weight, stacked.
MMS = {
    "fwc":    ("W_fwc", {"s4": 0, "cond": 164, "pred": 244, "prev": 288},
               [("C0", 128), ("C1", 116), ("C2", 128)],
               [[(0, 128)], [(128, 192)]]),
    "glufwc": ("W_fwc_glu", {"t": 0},
               [("T0", 128), ("T1", 64)],
               [[(0, 128)], [(128, 192)]]),
    "pg":     ("W_pg", {"fwc0": 0},
               [("F0", 128), ("F1", 64)],
               [[(0, 4)]]),
    "gru1i":  ("W1_ih", {"fwc0": 0, "pgf0": 192, "prev": 232},
               [("F0", 128), ("F1", 128), ("C2", 128)],
               [[(0, 128)], [(160, 288)], [(320, 448)],
                [(128, 160), (288, 320)], [(448, 480)]]),
    "gru1h":  ("W1_hh", {"s1": 0},
               [("S1a", 128), ("S1b", 32)],
               [[(0, 128)], [(160, 288)], [(320, 448)],
                [(128, 160), (288, 320)], [(448, 480)]]),
    "glu1":   ("Wg1", {"g1": 0},
               [("G1a", 128), ("G1b", 32)],
               [[(0, 128)], [(128, 160)]]),
    "gru2i":  ("W2_ih", {"o1": 0, "pgf1": 160, "prev": 200},
               [("O1a", 128), ("O1b", 128), ("C2", 128)],
               [[(0, 128)], [(128, 256)], [(256, 384)]]),
    "gru2h":  ("W2_hh", {"s2": 0},
               [("S2", 128)],
               [[(0, 128)], [(128, 256)], [(256, 384)]]),
    "glu2":   ("Wg2", {"g2": 0}, [("G2", 128)], [[(0, 128)]]),
    "gru3i":  ("W3_ih", {"o2": 0, "pgf2": 128, "prev": 168},
               [("O2", 128), ("T1", 128), ("C2", 128)],
               [[(0, 128)], [(128, 256)], [(256, 384)]]),
    "gru3h":  ("W3_hh", {"s3": 0},
               [("S3", 128)],
               [[(0, 128)], [(128, 256)], [(256, 384)]]),
    "glu3":   ("Wg3", {"g3": 0}, [("G3", 128)], [[(0, 128)]]),
    "skip":   ("W_skip", {"o1": 0, "o2": 160, "o3": 288, "fwc0": 416,
                          "pgf3": 608, "prev": 648},
               [("O1a", 128), ("O1b", 128), ("O2", 128), ("O3", 128),
                ("F0", 128), ("F1", 128), ("C2", 128), ("P3", 40)],
               [[(0, 128)]]),
    "glusk":  ("Wg_skip", {"kt": 0}, [("K0", 128)], [[(0, 128)]]),
    "sig":    ("W_sig", {"v": 0}, [("V0", 128)], [[(0, 40)]]),
}


def build_blob_layout():
    """Column layout of the big bf16 weight blob [128, TOT].
    Returns ({(mm, mb_idx, kc_idx): (off, K, M)}, total_cols)."""
    off = 0
    layout = {}
    for name, (_, _, kchunks, mblocks) in MMS.items():
        for mi, rows in enumerate(mblocks):
            M = sum(hi - lo for lo, hi in rows)
            for ki in range(len(kchunks)):
                K = kchunks[ki][1]
                layout[(name, mi, ki)] = (off, K, M)
                off += M
    # L-broadcast selection mats for pitch gains: [4, 128]; pg pairs at rows 0:40 / 64:104
    for k in range(2):
        layout[(f"Lp{k}", 0, 0)] = (off, 4, 128); off += 128
    return layout, off


BLOB_LAYOUT, BLOB_COLS = build_blob_layout()


def build_blob(w):
    """w: dict of effective fp32 weight arrays. Returns [128, BLOB_COLS] bf16."""
    blob = np.zeros((P, BLOB_COLS), np.float32)
    for name, (wkey, inmap, kchunks, mblocks) in MMS.items():
        Weff = np.asarray(w[wkey], np.float32)
        for mi, rows in enumerate(mblocks):
            Wrows = np.concatenate([Weff[lo:hi] for lo, hi in rows], 0)  # [M, D]
            for ki, (kc, K) in enumerate(kchunks):
                off, _, M = BLOB_LAYOUT[(name, mi, ki)]
                blk = np.zeros((P, M), np.float32)
                for src, lo, hi, plo in CHUNKS[kc]:
                    if src in inmap and plo < K:
                        d0 = inmap[src]
                        blk[plo:plo + (hi - lo)] = Wrows[:, d0 + lo:d0 + hi].T
                blob[:, off:off + M] = blk
    for k in range(2):
        off, _, _ = BLOB_LAYOUT[(f"Lp{k}", 0, 0)]
        L = np.zeros((P, 128), np.float32)
        L[2 * k, 0:40] = 1.0
        L[2 * k + 1, 64:104] = 1.0
        blob[:, off:off + 128] = L
    return blob.astype(ml_dtypes.bfloat16)


# batch-major staging column layout (bf16 stage [128, RG, 896])
ST = {
    "s4a": (0, 128), "cond": (128, 208), "s4b": (208, 244), "pad0": (244, 256),
    "fpitch": (256, 296), "pred": (296, 340), "prev": (340, 380), "pad1": (380, 384),
    "s1a": (384, 512), "s1b": (512, 544), "pad2": (544, 640),
    "s2": (640, 768), "s3": (768, 896),
}
ST_COLS = 896
# transpose source col -> destination chunk
ST_TRANSPOSE = [(0, "C0"), (128, "C1"), (256, "C2"), (384, "S1a"),
                (512, "S1b"), (640, "S2"), (768, "S3")]


def build_nc(n_bt=NBT):
    nc = bacc.Bacc("TRN2")
    rows = n_bt * NB

    # ---- DRAM I/O ----
    d_cond = nc.dram_tensor("cond_d", [rows, 80], F32, kind="ExternalInput")
    d_s1 = nc.dram_tensor("s1_d", [rows, 160], F32, kind="ExternalInput")
    d_s2 = nc.dram_tensor("s2_d", [rows, 128], F32, kind="ExternalInput")
    d_s3 = nc.dram_tensor("s3_d", [rows, 128], F32, kind="ExternalInput")
    d_s4 = nc.dram_tensor("s4_d", [rows, 164], F32, kind="ExternalInput")
    d_exc2d = nc.dram_tensor("exc2d_d", [rows, 256], F32, kind="ExternalInput")
    excn = EXC_PAD + rows * 256 + EXC_PAD
    d_excf = nc.dram_tensor("excf_d", [excn, 1], F32, kind="ExternalInput")
    d_ppin = nc.dram_tensor("ppin_d", [rows, 216], F32, kind="ExternalInput")
    d_pidx = nc.dram_tensor("pidx_d", [rows, 2], I32, kind="ExternalInput")
    d_blob = nc.dram_tensor("blob_d", [P, BLOB_COLS], BF16, kind="ExternalInput")
    d_wcg = nc.dram_tensor("wcg_d", [P, 80], F32, kind="ExternalInput")
    d_iota2 = nc.dram_tensor("iota2_d", [P, RG * 44], F32, kind="ExternalInput")
    d_bcg = nc.dram_tensor("bcg_d", [P, 1], F32, kind="ExternalInput")
    d_bpg = nc.dram_tensor("bpg_d", [4, 1], F32, kind="ExternalInput")

    o_sig = nc.dram_tensor("sig_o", [rows, 40], F32, kind="ExternalOutput")
    o_excn = nc.dram_tensor("excn_o", [rows, 256], F32, kind="ExternalOutput")
    o_ppn = nc.dram_tensor("ppn_o", [rows, 256], F32, kind="ExternalOutput")
    o_g1 = nc.dram_tensor("g1_o", [rows, 160], F32, kind="ExternalOutput")
    o_g2 = nc.dram_tensor("g2_o", [rows, 128], F32, kind="ExternalOutput")
    o_g3 = nc.dram_tensor("g3_o", [rows, 128], F32, kind="ExternalOutput")
    o_s4n = nc.dram_tensor("s4n_o", [rows, 164], F32, kind="ExternalOutput")

    def btview(t, bt, csl=None):
        v = t[bt * NB:(bt + 1) * NB, :] if csl is None else t[bt * NB:(bt + 1) * NB, csl]
        return v.rearrange("(g p) c -> p g c", p=P)

    wof = {}
    with tile.TileContext(nc) as tc:
        with ExitStack() as ctx:
            wp = ctx.enter_context(tc.tile_pool(name="wp", bufs=1))
            inp = ctx.enter_context(tc.tile_pool(name="inp", bufs=2))
            stp = ctx.enter_context(tc.tile_pool(name="stp", bufs=2))
            gap = ctx.enter_context(tc.tile_pool(name="gap", bufs=2))
            fmp = ctx.enter_context(tc.tile_pool(name="fmp", bufs=3))
            fm2 = ctx.enter_context(tc.tile_pool(name="fm2", bufs=2))
            mid = ctx.enter_context(tc.tile_pool(name="mid", bufs=2))
            gop = ctx.enter_context(tc.tile_pool(name="gop", bufs=2))
            ps5 = ctx.enter_context(tc.tile_pool(name="ps5", bufs=2, space="PSUM"))
            psA = ctx.enter_context(tc.tile_pool(name="psA", bufs=3, space="PSUM"))

            # ---- constants ----
            wsb = wp.tile([P, BLOB_COLS], BF16)
            nc.sync.dma_start(wsb[:], d_blob[:])
            identF = wp.tile([P, P], F32)
            make_identity(nc, identF)
            wcg = wp.tile([P, 80], F32)
            nc.sync.dma_start(wcg[:], d_wcg[:])
            iota2 = wp.tile([P, RG, 44], F32)
            nc.sync.dma_start(iota2[:], d_iota2[:].rearrange("p (g j) -> p g j", g=RG))
            bcg = wp.tile([P, 1], F32)
            nc.sync.dma_start(bcg[:], d_bcg[:])
            bpg = wp.tile([P, 1], F32)
            nc.sync.dma_start(bpg[0:4, :], d_bpg[:])

            def wsl(name, mi, ki):
                off, K, M = BLOB_LAYOUT[(name, mi, ki)]
                return wsb[0:K, off:off + M]

            heads = {}

            def emit_headA(bt):
                H = {}
                # ============ loads ============
                s4f = inp.tile([P, RG, 164], F32, tag="s4f")
                nc.sync.dma_start(s4f[:], btview(d_s4, bt))
                condf = inp.tile([P, RG, 80], F32, tag="condf")
                nc.sync.dma_start(condf[:], btview(d_cond, bt))
                s1f = inp.tile([P, RG, 160], F32, tag="s1f")
                nc.sync.dma_start(s1f[:], btview(d_s1, bt))
                s2f = inp.tile([P, RG, 128], F32, tag="s2f")
                nc.sync.dma_start(s2f[:], btview(d_s2, bt))
                s3f = inp.tile([P, RG, 128], F32, tag="s3f")
                nc.sync.dma_start(s3f[:], btview(d_s3, bt))
                exctf = inp.tile([P, RG, 40], F32, tag="exctf")
                nc.sync.dma_start(exctf[:], btview(d_exc2d, bt, slice(216, 256)))
                pidx = inp.tile([P, RG, 2], I32, tag="pidx")
                nc.sync.dma_start(pidx[:], btview(d_pidx, bt))

                # ============ DRAM->DRAM passthrough copies ============
                rsl = slice(bt * NB, (bt + 1) * NB)
                nc.sync.dma_start(o_excn[rsl, 0:216], d_exc2d[rsl, 40:256])
                nc.sync.dma_start(o_ppn[rsl, 0:216], d_ppin[rsl, :])

                # ============ pitch gather + fixups ============
                idxt = gap.tile([P, RG, 2], I32, tag="idxt")
                # idxA = base + 254 - period ; idxB = base + max(254 - 2*period, 0)
                t1 = gap.tile([P, RG, 1], I32, tag="t1")
                nc.gpsimd.tensor_scalar(t1[:], pidx[:, :, 0:1], -1, 254, op0=ALU.mult, op1=ALU.add)
                nc.gpsimd.tensor_tensor(idxt[:, :, 0:1], t1[:], pidx[:, :, 1:2], op=ALU.add)
                t2 = gap.tile([P, RG, 1], I32, tag="t2")
                nc.gpsimd.tensor_scalar(t2[:], pidx[:, :, 0:1], -2, 254, op0=ALU.mult, op1=ALU.add)
                nc.gpsimd.tensor_scalar(t2[:], t2[:], 0, None, op0=ALU.max)
                nc.gpsimd.tensor_tensor(idxt[:, :, 1:2], t2[:], pidx[:, :, 1:2], op=ALU.add)

                gA = gap.tile([P, RG, 44], F32, tag="gA")
                gB = gap.tile([P, RG, 44], F32, tag="gB")
                for rg in range(RG):
                    nc.gpsimd.indirect_dma_start(
                        out=gA[:, rg, :], out_offset=None, in_=d_excf[:, :],
                        in_offset=bass.IndirectOffsetOnAxis(ap=idxt[:, rg, 0:1], axis=0))
                    nc.gpsimd.indirect_dma_start(
                        out=gB[:, rg, :], out_offset=None, in_=d_excf[:, :],
                        in_offset=bass.IndirectOffsetOnAxis(ap=idxt[:, rg, 1:2], axis=0))

                H.update(condf=condf, exctf=exctf, pidx=pidx, gA=gA, gB=gB,
                         s4f=s4f, s1f=s1f, s2f=s2f, s3f=s3f)
                return H

            def emit_headB(bt, H):
                condf = H["condf"]; exctf = H["exctf"]; pidx = H["pidx"]
                gA = H["gA"]; gB = H["gB"]
                s4f = H["s4f"]; s1f = H["s1f"]; s2f = H["s2f"]; s3f = H["s3f"]

                def brc(t, n):
                    return bass.AP(tensor=t.tensor, offset=t.offset,
                                   ap=[list(t.ap[0]), list(t.ap[1]), [0, n]])

                # ============ gain chain (fp32, batch-major) ============
                gdot = gap.tile([P, RG, 80], F32, tag="gdot")
                wcg_b = bass.AP(tensor=wcg.tensor, offset=wcg.offset,
                                ap=[list(wcg.ap[0]), [0, RG], [1, 80]])
                nc.vector.tensor_tensor(gdot[:], condf[:], wcg_b, op=ALU.mult)
                gsum = gap.tile([P, RG, 1], F32, tag="gsum")
                nc.vector.tensor_reduce(gsum[:], gdot[:], axis=AX.X, op=ALU.add)
                gsig = gap.tile([P, RG, 1], F32, tag="gsig")
                nc.scalar.activation(gsig[:], gsum[:], AF.Sigmoid, bias=bcg[:, 0:1])
                gain = gap.tile([P, RG, 1], F32, tag="gain")
                nc.vector.tensor_scalar(gain[:], gsig[:], 0.8, 0.2, op0=ALU.mult, op1=ALU.add)
                gain2 = gap.tile([P, RG, 1], F32, tag="gain2")
                nc.vector.tensor_scalar(gain2[:], gain[:], 0.001, 20.0, op0=ALU.max, op1=ALU.min)
                gainp = gap.tile([P, RG, 1], F32, tag="gainp")
                nc.vector.tensor_scalar(gainp[:], gain2[:], 1e-5, None, op0=ALU.add)
                grec = gap.tile([P, RG, 1], F32, tag="grec")
                nc.vector.reciprocal(grec[:], gainp[:])

                perf = gap.tile([P, RG, 1], F32, tag="perf")
                nc.gpsimd.tensor_copy(perf[:], pidx[:, :, 0:1])
                mc = gap.tile([P, RG, 1], F32, tag="mc")
                nc.gpsimd.tensor_scalar(mc[:], perf[:], 255.0, None, op0=ALU.is_equal)
                wm = gap.tile([P, RG, 44], F32, tag="wm")
                nc.vector.tensor_tensor(wm[:], iota2[:], brc(perf, 44), op=ALU.is_ge)
                praw = gap.tile([P, RG, 44], F32, tag="praw")
                nc.vector.select(praw[:], wm[:].bitcast(I32), gB[:], gA[:])
                nc.vector.select(praw[:, :, 0:1], mc[:].bitcast(I32),
                                 gA[:, :, 1:2], gA[:, :, 0:1])
                predf = gap.tile([P, RG, 44], F32, tag="predf")
                nc.vector.tensor_tensor(predf[:], praw[:], brc(grec, 44), op=ALU.mult)
                prevf = gap.tile([P, RG, 40], F32, tag="prevf")
                nc.vector.tensor_tensor(prevf[:], exctf[:], brc(grec, 40), op=ALU.mult)

                # ============ cast into bf16 stage (chunk-major) + pad memsets ====
                st = stp.tile([P, 7, RG, 128], BF16, tag="stage")
                nc.gpsimd.memset(st[:, 1, :, 116:128], 0.0)
                nc.gpsimd.memset(st[:, 2, :, 124:128], 0.0)
                nc.gpsimd.memset(st[:, 4, :, 32:128], 0.0)
                nc.vector.tensor_copy(st[:, 0, :, :], s4f[:, :, 0:128])
                nc.vector.tensor_copy(st[:, 1, :, 0:80], condf[:])
                nc.vector.tensor_copy(st[:, 1, :, 80:116], s4f[:, :, 128:164])
                nc.vector.tensor_copy(st[:, 2, :, 0:40], predf[:, :, 2:42])
                nc.vector.tensor_copy(st[:, 2, :, 40:84], predf[:])
                nc.vector.tensor_copy(st[:, 2, :, 84:124], prevf[:])
                nc.vector.tensor_copy(st[:, 3, :, :], s1f[:, :, 0:128])
                nc.vector.tensor_copy(st[:, 4, :, 0:32], s1f[:, :, 128:160])
                nc.vector.tensor_copy(st[:, 5, :, :], s2f[:])
                nc.vector.tensor_copy(st[:, 6, :, :], s3f[:])

                # ============ transpose to feature-major chunks (blocked) ========
                ck = {}
                for name in ("C0", "C1", "C2", "S1a", "S1b", "S2", "S3", "T0", "T1",
                             "F0", "F1", "P3"):
                    ck[name] = fmp.tile([P, NB], BF16, tag=name, name=f"ck_{name}")
                for name in ("G1a", "O1a", "O1b", "G2", "O2", "G3", "O3", "K0", "V0"):
                    ck[name] = fm2.tile([P, NB], BF16, tag=name, name=f"ck_{name}")
                ck["G1b"] = fm2.tile([P, NB], BF16, tag="G1b", name="ck_G1b")
                for ci, dst in enumerate(("C0", "C1", "C2", "S1a", "S1b", "S2", "S3")):
                    eng = nc.sync
                    eng.dma_start_transpose(
                        ck[dst][:, :].rearrange("p (g c) -> p g c", g=RG),
                        st[:, ci, :, :])
                # junk-row memsets for chunks with partially-written tails
                nc.gpsimd.memset(ck["F1"][96:128, :], 0.0)
                nc.gpsimd.memset(ck["T1"][96:128, :], 0.0)
                nc.gpsimd.memset(ck["O1b"][32:64, :], 0.0)
                nc.gpsimd.memset(ck["O1b"][96:128, :], 0.0)
                H.update(ck=ck, condf=condf, predf=predf, prevf=prevf, gain2=gain2)
                return H

            def emit_body(bt, H):
                ck = H["ck"]; condf = H["condf"]; predf = H["predf"]
                prevf = H["prevf"]; gain2 = H["gain2"]

                def mm_acc(name, mi, targets, extra=None, stop=True):
                    """Emit K-accumulation for mblock mi of matmul `name` into psum AP."""
                    _, _, kchunks, _ = MMS[name]
                    ins = []
                    for ki, (kc, K) in enumerate(kchunks):
                        ins.append((wsl(name, mi, ki), ck[kc][0:K, :]))
                    return ins

                def emit(psum_ap, parts, starts=True, stops=True):
                    n = len(parts)
                    for i, (w, r) in enumerate(parts):
                        nc.tensor.matmul(psum_ap, w, r,
                                         start=(starts and i == 0),
                                         stop=(stops and i == n - 1))

                # ============ GRU1 hh (state-only deps) emitted early ============
                A1 = psA.tile([P, 2 * NB], F32, tag="ps1024")
                NB1 = psA.tile([P, 2 * NB], F32, tag="ps1024")
                emit(A1[:, 0:NB], mm_acc("gru1h", 0, None), stops=False)
                emit(A1[:, NB:2 * NB], mm_acc("gru1h", 1, None), stops=False)
                emit(NB1[:, NB:2 * NB], mm_acc("gru1h", 2, None))

                # ============ FWConv ============
                psF0 = ps5.tile([P, NB], F32, tag="ps512")
                psF1 = ps5.tile([P, NB], F32, tag="ps512")
                emit(psF0[:], mm_acc("fwc", 0, None))
                emit(psF1[0:64, :], mm_acc("fwc", 1, None))
                nc.scalar.activation(ck["T0"][:], psF0[:], AF.Tanh)
                nc.scalar.activation(ck["T1"][0:64, :], psF1[0:64, :], AF.Tanh)
                psU = psA.tile([P, 2 * NB], F32, tag="ps1024")
                nc.vector.memset(psU[64:128, NB:2 * NB][0:64, :], 0.0)
                emit(psU[:, 0:NB], mm_acc("glufwc", 0, None))
                emit(psU[0:64, NB:2 * NB], mm_acc("glufwc", 1, None))
                uu = mid.tile([P, 2 * NB], BF16, tag="uu")
                nc.scalar.activation(uu[:], psU[:], AF.Sigmoid)
                nc.vector.tensor_tensor(ck["F0"][:], ck["T0"][:], uu[:, 0:NB], op=ALU.mult)
                nc.vector.tensor_tensor(ck["F1"][0:64, :], ck["T1"][0:64, :],
                                        uu[0:64, NB:2 * NB], op=ALU.mult)

                # ============ pitch gains ============
                psPG = ps5.tile([P, NB], F32, tag="ps512")
                emit(psPG[0:4, :], mm_acc("pg", 0, None))
                pgs = mid.tile([P, NB], BF16, tag="pgs")
                nc.scalar.activation(pgs[0:4, :], psPG[0:4, :], AF.Sigmoid, bias=bpg[0:4, 0:1])
                psPa = ps5.tile([P, NB], F32, tag="ps512")
                psPb = ps5.tile([P, NB], F32, tag="ps512")
                nc.tensor.matmul(psPa[0:104, :], wsl("Lp0", 0, 0)[:, 0:104], pgs[0:4, :], start=True, stop=True)
                nc.tensor.matmul(psPb[0:104, :], wsl("Lp1", 0, 0)[:, 0:104], pgs[0:4, :], start=True, stop=True)
                nc.vector.tensor_tensor(ck["F1"][64:104, :], psPa[0:40, :], ck["C2"][0:40, :], op=ALU.mult)
                nc.vector.tensor_tensor(ck["O1b"][64:104, :], psPa[64:104, :], ck["C2"][0:40, :], op=ALU.mult)
                nc.vector.tensor_tensor(ck["T1"][64:104, :], psPb[0:40, :], ck["C2"][0:40, :], op=ALU.mult)
                nc.vector.tensor_tensor(ck["P3"][0:40, :], psPb[64:104, :], ck["C2"][0:40, :], op=ALU.mult)

                # ============ GRU1 (gates 160 = 128 + 32) ============
                emit(A1[:, 0:NB], mm_acc("gru1i", 0, None), starts=False)
                emit(A1[:, NB:2 * NB], mm_acc("gru1i", 1, None), starts=False)
                emit(NB1[:, 0:NB], mm_acc("gru1i", 2, None))
                TLt = ps5.tile([P, NB], F32, tag="ps512")
                B1t = ps5.tile([P, NB], F32, tag="ps512")
                TL = TLt[:, :]
                B1b = B1t[0:32, :]
                emit(TL[0:64, :], mm_acc("gru1i", 3, None), stops=False)
                emit(TL[0:64, :], mm_acc("gru1h", 3, None), starts=False)
                emit(TL[64:96, :], mm_acc("gru1i", 4, None))
                emit(B1b, mm_acc("gru1h", 4, None))

                # main (128-row) gates + tail (32-row) gates, tanh merged into one wide op
                rz = mid.tile([P, 2 * NB], BF16, tag="rz128")
                nc.scalar.activation(rz[:], A1[:], AF.Sigmoid)
                rz1t = mid.tile([P, NB], BF16, tag="rz1t")
                nc.scalar.activation(rz1t[0:64, :], TL[0:64, :], AF.Sigmoid)
                rh = mid.tile([P, NB], BF16, tag="rh128")
                nc.vector.tensor_tensor(rh[:], rz[:, 0:NB], NB1[:, NB:2 * NB], op=ALU.mult)
                rh1 = mid.tile([P, NB], BF16, tag="rh1t")
                nc.vector.tensor_tensor(rh1[0:32, :], rz1t[0:32, :], B1b, op=ALU.mult)
                np12 = mid.tile([P, 2 * NB], BF16, tag="np12")
                nc.vector.memset(np12[:, NB:2 * NB], 0.0)
                nc.vector.tensor_tensor(np12[:, 0:NB], NB1[:, 0:NB], rh[:], op=ALU.add)
                nc.vector.tensor_tensor(np12[0:32, NB:2 * NB], TL[64:96, :], rh1[0:32, :], op=ALU.add)
                nt12 = mid.tile([P, 2 * NB], BF16, tag="nt12")
                nc.scalar.activation(nt12[:], np12[:], AF.Tanh)
                sub = mid.tile([P, NB], BF16, tag="sb128")
                nc.vector.tensor_tensor(sub[:], ck["S1a"][:], nt12[:, 0:NB], op=ALU.subtract)
                zt = mid.tile([P, NB], BF16, tag="zt128")
                nc.vector.tensor_tensor(zt[:], rz[:, NB:2 * NB], sub[:], op=ALU.mult)
                nc.vector.tensor_tensor(ck["G1a"][:], nt12[:, 0:NB], zt[:], op=ALU.add)
                sbt = mid.tile([P, NB], BF16, tag="sbt")
                nc.vector.tensor_tensor(sbt[32:64, :], ck["S1b"][0:32, :],
                                        nt12[0:32, NB:2 * NB], op=ALU.subtract)
                zt1 = mid.tile([P, NB], BF16, tag="zt1t")
                nc.vector.tensor_tensor(zt1[0:32, :], rz1t[32:64, :], sbt[32:64, :], op=ALU.mult)
                nc.vector.tensor_tensor(ck["G1b"][0:32, :], nt12[0:32, NB:2 * NB], zt1[0:32, :], op=ALU.add)

                # ============ GLU1 ============
                U1 = psA.tile([P, 2 * NB], F32, tag="ps1024")
                nc.vector.memset(U1[32:128, NB:2 * NB][0:32, :], 0.0)
                nc.vector.memset(U1[64:128, NB:2 * NB], 0.0)
                emit(U1[:, 0:NB], mm_acc("glu1", 0, None))
                emit(U1[0:32, NB:2 * NB], mm_acc("glu1", 1, None))
                u1w = mid.tile([P, 2 * NB], BF16, tag="u1w")
                nc.scalar.activation(u1w[:], U1[:], AF.Sigmoid)
                nc.vector.tensor_tensor(ck["O1a"][:], ck["G1a"][:], u1w[:, 0:NB], op=ALU.mult)
                nc.vector.tensor_tensor(ck["O1b"][0:32, :], ck["G1b"][0:32, :],
                                        u1w[0:32, NB:2 * NB], op=ALU.mult)

                def gru_combine(Arz, NBw, s_chunk, g_out, width=P):
                    rz = mid.tile([width, 2 * NB], BF16, tag=f"rz{width}")
                    nc.scalar.activation(rz[:], Arz[:], AF.Sigmoid)
                    rh = mid.tile([width, NB], BF16, tag=f"rh{width}")
                    nc.vector.tensor_tensor(rh[:], rz[:, 0:NB], NBw[:, NB:2 * NB], op=ALU.mult)
                    npre = mid.tile([width, NB], BF16, tag=f"np{width}")
                    nc.vector.tensor_tensor(npre[:], NBw[:, 0:NB], rh[:], op=ALU.add)
                    nt = mid.tile([width, NB], BF16, tag=f"nt{width}")
                    nc.scalar.activation(nt[:], npre[:], AF.Tanh)
                    sub = mid.tile([width, NB], BF16, tag=f"sb{width}")
                    nc.vector.tensor_tensor(sub[:], s_chunk, nt[:], op=ALU.subtract)
                    zt = mid.tile([width, NB], BF16, tag=f"zt{width}")
                    nc.vector.tensor_tensor(zt[:], rz[:, NB:2 * NB], sub[:], op=ALU.mult)
                    nc.vector.tensor_tensor(g_out, nt[:], zt[:], op=ALU.add)

                # ============ GRU2 / GLU2 ============
                A2 = psA.tile([P, 2 * NB], F32, tag="ps1024")
                NB2 = psA.tile([P, 2 * NB], F32, tag="ps1024")
                emit(A2[:, 0:NB], mm_acc("gru2h", 0, None) + mm_acc("gru2i", 0, None))
                emit(A2[:, NB:2 * NB], mm_acc("gru2h", 1, None) + mm_acc("gru2i", 1, None))
                emit(NB2[:, NB:2 * NB], mm_acc("gru2h", 2, None))
                emit(NB2[:, 0:NB], mm_acc("gru2i", 2, None))
                gru_combine(A2, NB2, ck["S2"][:], ck["G2"][:], P)
                U2 = ps5.tile([P, NB], F32, tag="ps512")
                emit(U2[:], mm_acc("glu2", 0, None))
                u2 = mid.tile([P, NB], BF16, tag="u2")
                nc.scalar.activation(u2[:], U2[:], AF.Sigmoid)
                nc.vector.tensor_tensor(ck["O2"][:], ck["G2"][:], u2[:], op=ALU.mult)

                # ============ GRU3 / GLU3 ============
                A3 = psA.tile([P, 2 * NB], F32, tag="ps1024")
                NB3 = psA.tile([P, 2 * NB], F32, tag="ps1024")
                emit(A3[:, 0:NB], mm_acc("gru3h", 0, None) + mm_acc("gru3i", 0, None))
                emit(A3[:, NB:2 * NB], mm_acc("gru3h", 1, None) + mm_acc("gru3i", 1, None))
                emit(NB3[:, NB:2 * NB], mm_acc("gru3h", 2, None))
                emit(NB3[:, 0:NB], mm_acc("gru3i", 2, None))
                gru_combine(A3, NB3, ck["S3"][:], ck["G3"][:], P)
                U3 = ps5.tile([P, NB], F32, tag="ps512")
                emit(U3[:], mm_acc("glu3", 0, None))
                u3 = mid.tile([P, NB], BF16, tag="u3")
                nc.scalar.activation(u3[:], U3[:], AF.Sigmoid)
                nc.vector.tensor_tensor(ck["O3"][:], ck["G3"][:], u3[:], op=ALU.mult)

                # ============ skip / sig ============
                SK = ps5.tile([P, NB], F32, tag="ps512")
                emit(SK[:], mm_acc("skip", 0, None))
                nc.scalar.activation(ck["K0"][:], SK[:], AF.Tanh)
                UK = ps5.tile([P, NB], F32, tag="ps512")
                emit(UK[:], mm_acc("glusk", 0, None))
                uk = mid.tile([P, NB], BF16, tag="uk")
                nc.scalar.activation(uk[:], UK[:], AF.Sigmoid)
                nc.vector.tensor_tensor(ck["V0"][:], ck["K0"][:], uk[:], op=ALU.mult)
                SG = ps5.tile([P, NB], F32, tag="ps512")
                emit(SG[0:40, :], mm_acc("sig", 0, None))
                sigf = mid.tile([P, NB], F32, tag="sigf")
                nc.scalar.activation(sigf[0:40, :], SG[0:40, :], AF.Tanh)

                # sig-out: transpose back to batch-major, multiply by gain
                sigbm = gop.tile([P, RG, 40], F32, tag="sigbm")
                for rg in range(RG):
                    psT = ps5.tile([P, NB], F32, tag="ps512")
                    nc.tensor.transpose(psT[:, 0:40], sigf[0:40, rg * P:(rg + 1) * P],
                                        identF[0:40, 0:40])
                    nc.vector.tensor_scalar(sigbm[:, rg, :], psT[:, 0:40],
                                            gain2[:, rg, 0:1], None, op0=ALU.mult)
                nc.sync.dma_start(btview(o_sig, bt), sigbm[:])
                nc.sync.dma_start(btview(o_excn, bt, slice(216, 256)), sigbm[:])

                # ============ g1/g2/g3 outputs (bf16 xbar transpose + upcast) ====
                gbm = gop.tile([P, RG, 416], BF16, tag="gbm")
                nc.sync.dma_start_transpose(gbm[:, :, 0:128], ck["G1a"][:, :])
                nc.scalar.dma_start_transpose(gbm[:, :, 128:160], ck["G1b"][0:32, :])
                nc.sync.dma_start_transpose(gbm[:, :, 160:288], ck["G2"][:, :])
                nc.scalar.dma_start_transpose(gbm[:, :, 288:416], ck["G3"][:, :])
                gf = gop.tile([P, RG, 416], F32, tag="gf")
                nc.vector.tensor_copy(gf[:], gbm[:])
                nc.sync.dma_start(btview(o_g1, bt), gf[:, :, 0:160])
                nc.sync.dma_start(btview(o_g2, bt), gf[:, :, 160:288])
                nc.sync.dma_start(btview(o_g3, bt), gf[:, :, 288:416])

                # ============ s4_new / pp_new tails ============
                nc.sync.dma_start(btview(o_s4n, bt, slice(0, 80)), condf[:])
                nc.sync.dma_start(btview(o_s4n, bt, slice(80, 124)), predf[:])
                nc.sync.dma_start(btview(o_s4n, bt, slice(124, 164)), prevf[:])
                nc.sync.dma_start(btview(o_ppn, bt, slice(216, 256)), predf[:, :, 2:42])

            for step in range(n_bt + 1):
                if step < n_bt:
                    heads[step] = emit_headA(step)
                if step >= 1:
                    emit_body(step - 1, heads.pop(step - 1))
                if step < n_bt:
                    emit_headB(step, heads[step])

    nc.compile()
    return nc


# ---------------------------------------------------------------------------
# host entry
# ---------------------------------------------------------------------------
_NC_CACHE = {}


def _get_nc(n_bt=NBT):
    if n_bt not in _NC_CACHE:
        _NC_CACHE[n_bt] = build_nc(n_bt)
    return _NC_CACHE[n_bt]


def make_in_map(ins, lo, hi):
    """Build the per-core input map for rows [lo, hi)."""
    rows = hi - lo
    exc = np.ascontiguousarray(ins["exc_mem"][lo:hi]).astype(np.float32, copy=False)
    excf = np.zeros((EXC_PAD + rows * 256 + EXC_PAD, 1), np.float32)
    excf[EXC_PAD:EXC_PAD + rows * 256, 0] = exc.ravel()
    pidx = np.zeros((rows, 2), np.int32)
    pidx[:, 0] = np.asarray(ins["period"][lo:hi]).astype(np.int32)
    pidx[:, 1] = EXC_PAD + np.arange(rows, dtype=np.int32) * 256
    iota2 = np.broadcast_to(np.arange(44, dtype=np.float32) - 2.0, (P, 4, 44))
    blob = build_blob(ins)
    return {
        "cond_d": np.ascontiguousarray(ins["cond"][lo:hi], np.float32),
        "s1_d": np.ascontiguousarray(ins["s1"][lo:hi], np.float32),
        "s2_d": np.ascontiguousarray(ins["s2"][lo:hi], np.float32),
        "s3_d": np.ascontiguousarray(ins["s3"][lo:hi], np.float32),
        "s4_d": np.ascontiguousarray(ins["s4"][lo:hi], np.float32),
        "exc2d_d": exc,
        "excf_d": excf,
        "ppin_d": np.ascontiguousarray(ins["prev_pred"][lo:hi, 40:256], np.float32),
        "pidx_d": pidx,
        "blob_d": blob,
        "wcg_d": np.broadcast_to(np.asarray(ins["W_cg"], np.float32).reshape(1, 80),
                                 (P, 80)).copy(),
        "iota2_d": np.ascontiguousarray(iota2.reshape(P, 176)),
        "bcg_d": np.full((P, 1), float(np.asarray(ins["b_cg"]).ravel()[0]), np.float32),
        "bpg_d": np.asarray(ins["b_pg"], np.float32).reshape(4, 1),
    }


def kernel(**inputs):
    from concourse.bass_utils import run_bass_kernel_spmd
    nc = _get_nc()
    in_maps = [make_in_map(inputs, c * R, (c + 1) * R) for c in range(N_CORES)]
    res = run_bass_kernel_spmd(nc, in_maps, core_ids=list(range(N_CORES)),
                               trace=bool(int(os.environ.get("K_TRACE", "0"))))
    outs = res.results
    if bool(int(os.environ.get("K_TRACE", "0"))):
        kernel.last_result = res
    cat = lambda k: np.concatenate([outs[c][k] for c in range(N_CORES)], 0)
    return (cat("sig_o"), cat("excn_o"), cat("ppn_o"),
            cat("g1_o"), cat("g2_o"), cat("g3_o"), cat("s4n_o"))


# revision 28
# speedup vs baseline: 1.0537x; 1.0537x over previous
"""FARGAN subframe step on 8 Trainium2 NeuronCores.

Strategy: pure data-parallel over batch (8192 rows/core). Feature-major
compute: activations live as bf16 [feature<=128, 512] SBUF chunks; every
matmul is out[Mchunk, 512] += lhsT.T @ rhs with lhsT = host-packed bf16
weight blocks (zero rows for features a chunk carries that a given matmul
doesn't consume). Batch-major <-> feature-major layout changes ride the
DMA transpose xbar (bf16). The pitch-lag gather uses indirect DMA with one
descriptor per row. Elementwise work is split across DVE / ACT / GPSIMD.
"""
import os
import numpy as np
import ml_dtypes

import concourse.bass as bass
from concourse import bacc
import concourse.tile as tile
import concourse.mybir as mybir
from concourse.masks import make_identity
from contextlib import ExitStack

F32 = mybir.dt.float32
BF16 = mybir.dt.bfloat16
I32 = mybir.dt.int32
AF = mybir.ActivationFunctionType
ALU = mybir.AluOpType
AX = mybir.AxisListType

P = 128
NB = 512               # batch tile (psum bank width in fp32)
RG = 4                 # row groups per batch tile
N_CORES = 8
R = 65536 // N_CORES   # rows per core
NBT = R // NB          # batch tiles per core
EXC_PAD = 64
EXCN = EXC_PAD + R * 256 + EXC_PAD

# ---------------------------------------------------------------------------
# feature-major chunk layout: chunk -> [(src, lo, hi, plo)]
# ---------------------------------------------------------------------------
CHUNKS = {
    "C0":  [("s4", 0, 128, 0)],
    "C1":  [("cond", 0, 80, 0), ("s4", 128, 164, 80)],
    "C2":  [("fpitch", 0, 40, 0), ("pred", 0, 44, 40), ("prev", 0, 40, 84)],
    "S1a": [("s1", 0, 128, 0)],
    "S1b": [("s1", 128, 160, 0)],
    "S2":  [("s2", 0, 128, 0)],
    "S3":  [("s3", 0, 128, 0)],
    "T0":  [("t", 0, 128, 0)],
    "T1":  [("t", 128, 192, 0), ("pgf2", 0, 40, 64)],
    "F0":  [("fwc0", 0, 128, 0)],
    "F1":  [("fwc0", 128, 192, 0), ("pgf0", 0, 40, 64)],
    "G1a": [("g1", 0, 128, 0)],
    "G1b": [("g1", 128, 160, 0)],
    "O1a": [("o1", 0, 128, 0)],
    "O1b": [("o1", 128, 160, 0), ("pgf1", 0, 40, 64)],
    "G2":  [("g2", 0, 128, 0)],
    "O2":  [("o2", 0, 128, 0)],
    "G3":  [("g3", 0, 128, 0)],
    "O3":  [("o3", 0, 128, 0)],
    "K0":  [("kt", 0, 128, 0)],
    "V0":  [("v", 0, 128, 0)],
    "P3":  [("pgf3", 0, 40, 0)],
}

# matmul table: name -> (weight key, inmap {src: dcol}, [(kchunk, K)], [mblock rows])
# mblock rows: list of (lo, hi) row ranges of the effective weight, stacked.
MMS = {
    "fwc":    ("W_fwc", {"s4": 0, "cond": 164, "pred": 244, "prev": 288},
               [("C0", 128), ("C1", 116), ("C2", 128)],
               [[(0, 128)], [(128, 192)]]),
    "glufwc": ("W_fwc_glu", {"t": 0},
               [("T0", 128), ("T1", 64)],
               [[(0, 128)], [(128, 192)]]),
    "pg":     ("W_pg", {"fwc0": 0},
               [("F0", 128), ("F1", 64)],
               [[(0, 4)]]),
    "gru1i":  ("W1_ih", {"fwc0": 0, "pgf0": 192, "prev": 232},
               [("F0", 128), ("F1", 128), ("C2", 128)],
               [[(0, 128)], [(160, 288)], [(320, 448)],
                [(128, 160), (288, 320)], [(448, 480)]]),
    "gru1h":  ("W1_hh", {"s1": 0},
               [("S1a", 128), ("S1b", 32)],
               [[(0, 128)], [(160, 288)], [(320, 448)],
                [(128, 160), (288, 320)], [(448, 480)]]),
    "glu1":   ("Wg1", {"g1": 0},
               [("G1a", 128), ("G1b", 32)],
               [[(0, 128)], [(128, 160)]]),
    "gru2i":  ("W2_ih", {"o1": 0, "pgf1": 160, "prev": 200},
               [("O1a", 128), ("O1b", 128), ("C2", 128)],
               [[(0, 128)], [(128, 256)], [(256, 384)]]),
    "gru2h":  ("W2_hh", {"s2": 0},
               [("S2", 128)],
               [[(0, 128)], [(128, 256)], [(256, 384)]]),
    "glu2":   ("Wg2", {"g2": 0}, [("G2", 128)], [[(0, 128)]]),
    "gru3i":  ("W3_ih", {"o2": 0, "pgf2": 128, "prev": 168},
               [("O2", 128), ("T1", 128), ("C2", 128)],
               [[(0, 128)], [(128, 256)], [(256, 384)]]),
    "gru3h":  ("W3_hh", {"s3": 0},
               [("S3", 128)],
               [[(0, 128)], [(128, 256)], [(256, 384)]]),
    "glu3":   ("Wg3", {"g3": 0}, [("G3", 128)], [[(0, 128)]]),
    "skip":   ("W_skip", {"o1": 0, "o2": 160, "o3": 288, "fwc0": 416,
                          "pgf3": 608, "prev": 648},
               [("O1a", 128), ("O1b", 128), ("O2", 128), ("O3", 128),
                ("F0", 128), ("F1", 128), ("C2", 128), ("P3", 40)],
               [[(0, 128)]]),
    "glusk":  ("Wg_skip", {"kt": 0}, [("K0", 128)], [[(0, 128)]]),
    "sig":    ("W_sig", {"v": 0}, [("V0", 128)], [[(0, 40)]]),
}


def build_blob_layout():
    """Column layout of the big bf16 weight blob [128, TOT].
    Returns ({(mm, mb_idx, kc_idx): (off, K, M)}, total_cols)."""
    off = 0
    layout = {}
    for name, (_, _, kchunks, mblocks) in MMS.items():
        for mi, rows in enumerate(mblocks):
            M = sum(hi - lo for lo, hi in rows)
            for ki in range(len(kchunks)):
                K = kchunks[ki][1]
                layout[(name, mi, ki)] = (off, K, M)
                off += M
    # L-broadcast selection mats for pitch gains: [4, 128]; pg pairs at rows 0:40 / 64:104
    for k in range(2):
        layout[(f"Lp{k}", 0, 0)] = (off, 4, 128); off += 128
    return layout, off


BLOB_LAYOUT, BLOB_COLS = build_blob_layout()


def build_blob(w):
    """w: dict of effective fp32 weight arrays. Returns [128, BLOB_COLS] bf16."""
    blob = np.zeros((P, BLOB_COLS), np.float32)
    for name, (wkey, inmap, kchunks, mblocks) in MMS.items():
        Weff = np.asarray(w[wkey], np.float32)
        for mi, rows in enumerate(mblocks):
            Wrows = np.concatenate([Weff[lo:hi] for lo, hi in rows], 0)  # [M, D]
            for ki, (kc, K) in enumerate(kchunks):
                off, _, M = BLOB_LAYOUT[(name, mi, ki)]
                blk = np.zeros((P, M), np.float32)
                for src, lo, hi, plo in CHUNKS[kc]:
                    if src in inmap and plo < K:
                        d0 = inmap[src]
                        blk[plo:plo + (hi - lo)] = Wrows[:, d0 + lo:d0 + hi].T
                blob[:, off:off + M] = blk
    for k in range(2):
        off, _, _ = BLOB_LAYOUT[(f"Lp{k}", 0, 0)]
        L = np.zeros((P, 128), np.float32)
        L[2 * k, 0:40] = 1.0
        L[2 * k + 1, 64:104] = 1.0
        blob[:, off:off + 128] = L
    return blob.astype(ml_dtypes.bfloat16)


# batch-major staging column layout (bf16 stage [128, RG, 896])
ST = {
    "s4a": (0, 128), "cond": (128, 208), "s4b": (208, 244), "pad0": (244, 256),
    "fpitch": (256, 296), "pred": (296, 340), "prev": (340, 380), "pad1": (380, 384),
    "s1a": (384, 512), "s1b": (512, 544), "pad2": (544, 640),
    "s2": (640, 768), "s3": (768, 896),
}
ST_COLS = 896
# transpose source col -> destination chunk
ST_TRANSPOSE = [(0, "C0"), (128, "C1"), (256, "C2"), (384, "S1a"),
                (512, "S1b"), (640, "S2"), (768, "S3")]


def build_nc(n_bt=NBT):
    nc = bacc.Bacc("TRN2")
    rows = n_bt * NB

    # ---- DRAM I/O ----
    d_cond = nc.dram_tensor("cond_d", [rows, 80], F32, kind="ExternalInput")
    d_s1 = nc.dram_tensor("s1_d", [rows, 160], F32, kind="ExternalInput")
    d_s2 = nc.dram_tensor("s2_d", [rows, 128], F32, kind="ExternalInput")
    d_s3 = nc.dram_tensor("s3_d", [rows, 128], F32, kind="ExternalInput")
    d_s4 = nc.dram_tensor("s4_d", [rows, 164], F32, kind="ExternalInput")
    d_exc2d = nc.dram_tensor("exc2d_d", [rows, 256], F32, kind="ExternalInput")
    excn = EXC_PAD + rows * 256 + EXC_PAD
    d_excf = nc.dram_tensor("excf_d", [excn, 1], F32, kind="ExternalInput")
    d_ppin = nc.dram_tensor("ppin_d", [rows, 216], F32, kind="ExternalInput")
    d_pidx = nc.dram_tensor("pidx_d", [rows, 2], I32, kind="ExternalInput")
    d_blob = nc.dram_tensor("blob_d", [P, BLOB_COLS], BF16, kind="ExternalInput")
    d_wcg = nc.dram_tensor("wcg_d", [P, 80], F32, kind="ExternalInput")
    d_iota2 = nc.dram_tensor("iota2_d", [P, RG * 44], F32, kind="ExternalInput")
    d_bcg = nc.dram_tensor("bcg_d", [P, 1], F32, kind="ExternalInput")
    d_bpg = nc.dram_tensor("bpg_d", [4, 1], F32, kind="ExternalInput")

    o_sig = nc.dram_tensor("sig_o", [rows, 40], F32, kind="ExternalOutput")
    o_excn = nc.dram_tensor("excn_o", [rows, 256], F32, kind="ExternalOutput")
    o_ppn = nc.dram_tensor("ppn_o", [rows, 256], F32, kind="ExternalOutput")
    o_g1 = nc.dram_tensor("g1_o", [rows, 160], F32, kind="ExternalOutput")
    o_g2 = nc.dram_tensor("g2_o", [rows, 128], F32, kind="ExternalOutput")
    o_g3 = nc.dram_tensor("g3_o", [rows, 128], F32, kind="ExternalOutput")
    o_s4n = nc.dram_tensor("s4n_o", [rows, 164], F32, kind="ExternalOutput")

    def btview(t, bt, csl=None):
        v = t[bt * NB:(bt + 1) * NB, :] if csl is None else t[bt * NB:(bt + 1) * NB, csl]
        return v.rearrange("(g p) c -> p g c", p=P)

    wof = {}
    with tile.TileContext(nc) as tc:
        with ExitStack() as ctx:
            wp = ctx.enter_context(tc.tile_pool(name="wp", bufs=1))
            inp = ctx.enter_context(tc.tile_pool(name="inp", bufs=2))
            stp = ctx.enter_context(tc.tile_pool(name="stp", bufs=2))
            gap = ctx.enter_context(tc.tile_pool(name="gap", bufs=2))
            fmp = ctx.enter_context(tc.tile_pool(name="fmp", bufs=3))
            fm2 = ctx.enter_context(tc.tile_pool(name="fm2", bufs=2))
            mid = ctx.enter_context(tc.tile_pool(name="mid", bufs=2))
            gop = ctx.enter_context(tc.tile_pool(name="gop", bufs=2))
            ps5 = ctx.enter_context(tc.tile_pool(name="ps5", bufs=2, space="PSUM"))
            psA = ctx.enter_context(tc.tile_pool(name="psA", bufs=3, space="PSUM"))

            # ---- constants ----
            wsb = wp.tile([P, BLOB_COLS], BF16)
            nc.sync.dma_start(wsb[:], d_blob[:])
            identF = wp.tile([P, P], F32)
            make_identity(nc, identF)
            wcg = wp.tile([P, 80], F32)
            nc.sync.dma_start(wcg[:], d_wcg[:])
            iota2 = wp.tile([P, RG, 44], F32)
            nc.sync.dma_start(iota2[:], d_iota2[:].rearrange("p (g j) -> p g j", g=RG))
            bcg = wp.tile([P, 1], F32)
            nc.sync.dma_start(bcg[:], d_bcg[:])
            bpg = wp.tile([P, 1], F32)
            nc.sync.dma_start(bpg[0:4, :], d_bpg[:])

            def wsl(name, mi, ki):
                off, K, M = BLOB_LAYOUT[(name, mi, ki)]
                return wsb[0:K, off:off + M]

            heads = {}

            def emit_headA(bt):
                H = {}
                # ============ loads ============
                s4f = inp.tile([P, RG, 164], F32, tag="s4f")
                nc.sync.dma_start(s4f[:], btview(d_s4, bt))
                condf = inp.tile([P, RG, 80], F32, tag="condf")
                nc.sync.dma_start(condf[:], btview(d_cond, bt))
                s1f = inp.tile([P, RG, 160], F32, tag="s1f")
                nc.sync.dma_start(s1f[:], btview(d_s1, bt))
                s2f = inp.tile([P, RG, 128], F32, tag="s2f")
                nc.sync.dma_start(s2f[:], btview(d_s2, bt))
                s3f = inp.tile([P, RG, 128], F32, tag="s3f")
                nc.sync.dma_start(s3f[:], btview(d_s3, bt))
                exctf = inp.tile([P, RG, 40], F32, tag="exctf")
                nc.sync.dma_start(exctf[:], btview(d_exc2d, bt, slice(216, 256)))
                pidx = inp.tile([P, RG, 2], I32, tag="pidx")
                nc.sync.dma_start(pidx[:], btview(d_pidx, bt))

                # ============ DRAM->DRAM passthrough copies ============
                rsl = slice(bt * NB, (bt + 1) * NB)
                nc.sync.dma_start(o_excn[rsl, 0:216], d_exc2d[rsl, 40:256])
                nc.sync.dma_start(o_ppn[rsl, 0:216], d_ppin[rsl, :])

                # ============ pitch gather + fixups ============
                idxt = gap.tile([P, RG, 2], I32, tag="idxt")
                # idxA = base + 254 - period ; idxB = base + max(254 - 2*period, 0)
                t1 = gap.tile([P, RG, 1], I32, tag="t1")
                nc.gpsimd.tensor_scalar(t1[:], pidx[:, :, 0:1], -1, 254, op0=ALU.mult, op1=ALU.add)
                nc.gpsimd.tensor_tensor(idxt[:, :, 0:1], t1[:], pidx[:, :, 1:2], op=ALU.add)
                t2 = gap.tile([P, RG, 1], I32, tag="t2")
                nc.gpsimd.tensor_scalar(t2[:], pidx[:, :, 0:1], -2, 254, op0=ALU.mult, op1=ALU.add)
                nc.gpsimd.tensor_scalar(t2[:], t2[:], 0, None, op0=ALU.max)
                nc.gpsimd.tensor_tensor(idxt[:, :, 1:2], t2[:], pidx[:, :, 1:2], op=ALU.add)

                gA = gap.tile([P, RG, 44], F32, tag="gA")
                gB = gap.tile([P, RG, 44], F32, tag="gB")
                for rg in range(RG):
                    nc.gpsimd.indirect_dma_start(
                        out=gA[:, rg, :], out_offset=None, in_=d_excf[:, :],
                        in_offset=bass.IndirectOffsetOnAxis(ap=idxt[:, rg, 0:1], axis=0))
                    nc.gpsimd.indirect_dma_start(
                        out=gB[:, rg, :], out_offset=None, in_=d_excf[:, :],
                        in_offset=bass.IndirectOffsetOnAxis(ap=idxt[:, rg, 1:2], axis=0))

                H.update(condf=condf, exctf=exctf, pidx=pidx, gA=gA, gB=gB,
                         s4f=s4f, s1f=s1f, s2f=s2f, s3f=s3f)
                return H

            def emit_headB(bt, H):
                condf = H["condf"]; exctf = H["exctf"]; pidx = H["pidx"]
                gA = H["gA"]; gB = H["gB"]
                s4f = H["s4f"]; s1f = H["s1f"]; s2f = H["s2f"]; s3f = H["s3f"]

                def brc(t, n):
                    return bass.AP(tensor=t.tensor, offset=t.offset,
                                   ap=[list(t.ap[0]), list(t.ap[1]), [0, n]])

                # ============ gain chain (fp32, batch-major) ============
                gdot = gap.tile([P, RG, 80], F32, tag="gdot")
                wcg_b = bass.AP(tensor=wcg.tensor, offset=wcg.offset,
                                ap=[list(wcg.ap[0]), [0, RG], [1, 80]])
                nc.vector.tensor_tensor(gdot[:], condf[:], wcg_b, op=ALU.mult)
                gsum = gap.tile([P, RG, 1], F32, tag="gsum")
                nc.vector.tensor_reduce(gsum[:], gdot[:], axis=AX.X, op=ALU.add)
                gsig = gap.tile([P, RG, 1], F32, tag="gsig")
                nc.scalar.activation(gsig[:], gsum[:], AF.Sigmoid, bias=bcg[:, 0:1])
                gain = gap.tile([P, RG, 1], F32, tag="gain")
                nc.vector.tensor_scalar(gain[:], gsig[:], 0.8, 0.2, op0=ALU.mult, op1=ALU.add)
                gain2 = gap.tile([P, RG, 1], F32, tag="gain2")
                nc.vector.tensor_scalar(gain2[:], gain[:], 0.001, 20.0, op0=ALU.max, op1=ALU.min)
                gainp = gap.tile([P, RG, 1], F32, tag="gainp")
                nc.vector.tensor_scalar(gainp[:], gain2[:], 1e-5, None, op0=ALU.add)
                grec = gap.tile([P, RG, 1], F32, tag="grec")
                nc.vector.reciprocal(grec[:], gainp[:])

                perf = gap.tile([P, RG, 1], F32, tag="perf")
                nc.gpsimd.tensor_copy(perf[:], pidx[:, :, 0:1])
                mc = gap.tile([P, RG, 1], F32, tag="mc")
                nc.gpsimd.tensor_scalar(mc[:], perf[:], 255.0, None, op0=ALU.is_equal)
                wm = gap.tile([P, RG, 44], F32, tag="wm")
                nc.vector.tensor_tensor(wm[:], iota2[:], brc(perf, 44), op=ALU.is_ge)
                praw = gap.tile([P, RG, 44], F32, tag="praw")
                nc.vector.select(praw[:], wm[:].bitcast(I32), gB[:], gA[:])
                nc.vector.select(praw[:, :, 0:1], mc[:].bitcast(I32),
                                 gA[:, :, 1:2], gA[:, :, 0:1])
                predf = gap.tile([P, RG, 44], F32, tag="predf")
                nc.vector.tensor_tensor(predf[:], praw[:], brc(grec, 44), op=ALU.mult)
                prevf = gap.tile([P, RG, 40], F32, tag="prevf")
                nc.vector.tensor_tensor(prevf[:], exctf[:], brc(grec, 40), op=ALU.mult)

                # ============ cast into bf16 stage (chunk-major) + pad memsets ====
                st = stp.tile([P, 7, RG, 128], BF16, tag="stage")
                nc.gpsimd.memset(st[:, 1, :, 116:128], 0.0)
                nc.gpsimd.memset(st[:, 2, :, 124:128], 0.0)
                nc.gpsimd.memset(st[:, 4, :, 32:128], 0.0)
                nc.vector.tensor_copy(st[:, 0, :, :], s4f[:, :, 0:128])
                nc.vector.tensor_copy(st[:, 1, :, 0:80], condf[:])
                nc.vector.tensor_copy(st[:, 1, :, 80:116], s4f[:, :, 128:164])
                nc.vector.tensor_copy(st[:, 2, :, 0:40], predf[:, :, 2:42])
                nc.vector.tensor_copy(st[:, 2, :, 40:84], predf[:])
                nc.vector.tensor_copy(st[:, 2, :, 84:124], prevf[:])
                nc.vector.tensor_copy(st[:, 3, :, :], s1f[:, :, 0:128])
                nc.vector.tensor_copy(st[:, 4, :, 0:32], s1f[:, :, 128:160])
                nc.vector.tensor_copy(st[:, 5, :, :], s2f[:])
                nc.vector.tensor_copy(st[:, 6, :, :], s3f[:])

                # ============ transpose to feature-major chunks (blocked) ========
                ck = {}
                for name in ("C0", "C1", "C2", "S1a", "S1b", "S2", "S3", "T0", "T1",
                             "F0", "F1", "P3"):
                    ck[name] = fmp.tile([P, NB], BF16, tag=name, name=f"ck_{name}")
                for name in ("G1a", "O1a", "O1b", "G2", "O2", "G3", "O3", "K0", "V0"):
                    ck[name] = fm2.tile([P, NB], BF16, tag=name, name=f"ck_{name}")
                ck["G1b"] = fm2.tile([P, NB], BF16, tag="G1b", name="ck_G1b")
                for ci, dst in enumerate(("C0", "C1", "C2", "S1a", "S1b", "S2", "S3")):
                    eng = nc.sync
                    eng.dma_start_transpose(
                        ck[dst][:, :].rearrange("p (g c) -> p g c", g=RG),
                        st[:, ci, :, :])
                # junk-row memsets for chunks with partially-written tails
                nc.gpsimd.memset(ck["F1"][96:128, :], 0.0)
                nc.gpsimd.memset(ck["T1"][96:128, :], 0.0)
                nc.gpsimd.memset(ck["O1b"][32:64, :], 0.0)
                nc.gpsimd.memset(ck["O1b"][96:128, :], 0.0)
                H.update(ck=ck, condf=condf, predf=predf, prevf=prevf, gain2=gain2)
                return H

            def emit_body(bt, H):
                ck = H["ck"]; condf = H["condf"]; predf = H["predf"]
                prevf = H["prevf"]; gain2 = H["gain2"]

                def mm_acc(name, mi, targets, extra=None, stop=True):
                    """Emit K-accumulation for mblock mi of matmul `name` into psum AP."""
                    _, _, kchunks, _ = MMS[name]
                    ins = []
                    for ki, (kc, K) in enumerate(kchunks):
                        ins.append((wsl(name, mi, ki), ck[kc][0:K, :]))
                    return ins

                def emit(psum_ap, parts, starts=True, stops=True):
                    n = len(parts)
                    for i, (w, r) in enumerate(parts):
                        nc.tensor.matmul(psum_ap, w, r,
                                         start=(starts and i == 0),
                                         stop=(stops and i == n - 1))

                # ============ GRU1 hh (state-only deps) emitted early ============
                A1 = psA.tile([P, 2 * NB], F32, tag="ps1024")
                NB1 = psA.tile([P, 2 * NB], F32, tag="ps1024")
                emit(A1[:, 0:NB], mm_acc("gru1h", 0, None), stops=False)
                emit(A1[:, NB:2 * NB], mm_acc("gru1h", 1, None), stops=False)
                emit(NB1[:, NB:2 * NB], mm_acc("gru1h", 2, None))

                # ============ FWConv ============
                psF0 = ps5.tile([P, NB], F32, tag="ps512")
                psF1 = ps5.tile([P, NB], F32, tag="ps512")
                emit(psF0[:], mm_acc("fwc", 0, None))
                emit(psF1[0:64, :], mm_acc("fwc", 1, None))
                nc.scalar.activation(ck["T0"][:], psF0[:], AF.Tanh)
                nc.scalar.activation(ck["T1"][0:64, :], psF1[0:64, :], AF.Tanh)
                psU = psA.tile([P, 2 * NB], F32, tag="ps1024")
                nc.vector.memset(psU[64:128, NB:2 * NB][0:64, :], 0.0)
                emit(psU[:, 0:NB], mm_acc("glufwc", 0, None))
                emit(psU[0:64, NB:2 * NB], mm_acc("glufwc", 1, None))
                uu = mid.tile([P, 2 * NB], BF16, tag="uu")
                nc.scalar.activation(uu[:], psU[:], AF.Sigmoid)
                nc.vector.tensor_tensor(ck["F0"][:], ck["T0"][:], uu[:, 0:NB], op=ALU.mult)
                nc.vector.tensor_tensor(ck["F1"][0:64, :], ck["T1"][0:64, :],
                                        uu[0:64, NB:2 * NB], op=ALU.mult)

                # ============ pitch gains ============
                psPG = ps5.tile([P, NB], F32, tag="ps512")
                emit(psPG[0:4, :], mm_acc("pg", 0, None))
                pgs = mid.tile([P, NB], BF16, tag="pgs")
                nc.scalar.activation(pgs[0:4, :], psPG[0:4, :], AF.Sigmoid, bias=bpg[0:4, 0:1])
                psPa = ps5.tile([P, NB], F32, tag="ps512")
                psPb = ps5.tile([P, NB], F32, tag="ps512")
                nc.tensor.matmul(psPa[0:104, :], wsl("Lp0", 0, 0)[:, 0:104], pgs[0:4, :], start=True, stop=True)
                nc.tensor.matmul(psPb[0:104, :], wsl("Lp1", 0, 0)[:, 0:104], pgs[0:4, :], start=True, stop=True)
                nc.vector.tensor_tensor(ck["F1"][64:104, :], psPa[0:40, :], ck["C2"][0:40, :], op=ALU.mult)
                nc.vector.tensor_tensor(ck["O1b"][64:104, :], psPa[64:104, :], ck["C2"][0:40, :], op=ALU.mult)
                nc.vector.tensor_tensor(ck["T1"][64:104, :], psPb[0:40, :], ck["C2"][0:40, :], op=ALU.mult)
                nc.vector.tensor_tensor(ck["P3"][0:40, :], psPb[64:104, :], ck["C2"][0:40, :], op=ALU.mult)

                # ============ GRU1 (gates 160 = 128 + 32) ============
                emit(A1[:, 0:NB], mm_acc("gru1i", 0, None), starts=False)
                emit(A1[:, NB:2 * NB], mm_acc("gru1i", 1, None), starts=False)
                emit(NB1[:, 0:NB], mm_acc("gru1i", 2, None))
                TLt = ps5.tile([P, NB], F32, tag="ps512")
                B1t = ps5.tile([P, NB], F32, tag="ps512")
                TL = TLt[:, :]
                B1b = B1t[0:32, :]
                emit(TL[0:64, :], mm_acc("gru1i", 3, None), stops=False)
                emit(TL[0:64, :], mm_acc("gru1h", 3, None), starts=False)
                emit(TL[64:96, :], mm_acc("gru1i", 4, None))
                emit(B1b, mm_acc("gru1h", 4, None))

                # main (128-row) gates + tail (32-row) gates, tanh merged into one wide op
                rz = mid.tile([P, 2 * NB], BF16, tag="rz128")
                nc.scalar.activation(rz[:], A1[:], AF.Sigmoid)
                rz1t = mid.tile([P, NB], BF16, tag="rz1t")
                nc.scalar.activation(rz1t[0:64, :], TL[0:64, :], AF.Sigmoid)
                rh = mid.tile([P, NB], BF16, tag="rh128")
                nc.vector.tensor_tensor(rh[:], rz[:, 0:NB], NB1[:, NB:2 * NB], op=ALU.mult)
                rh1 = mid.tile([P, NB], BF16, tag="rh1t")
                nc.vector.tensor_tensor(rh1[0:32, :], rz1t[0:32, :], B1b, op=ALU.mult)
                np12 = mid.tile([P, 2 * NB], BF16, tag="np12")
                nc.vector.memset(np12[:, NB:2 * NB], 0.0)
                nc.vector.tensor_tensor(np12[:, 0:NB], NB1[:, 0:NB], rh[:], op=ALU.add)
                nc.vector.tensor_tensor(np12[0:32, NB:2 * NB], TL[64:96, :], rh1[0:32, :], op=ALU.add)
                nt12 = mid.tile([P, 2 * NB], BF16, tag="nt12")
                nc.scalar.activation(nt12[:], np12[:], AF.Tanh)
                sub = mid.tile([P, NB], BF16, tag="sb128")
                nc.vector.tensor_tensor(sub[:], ck["S1a"][:], nt12[:, 0:NB], op=ALU.subtract)
                zt = mid.tile([P, NB], BF16, tag="zt128")
                nc.vector.tensor_tensor(zt[:], rz[:, NB:2 * NB], sub[:], op=ALU.mult)
                nc.vector.tensor_tensor(ck["G1a"][:], nt12[:, 0:NB], zt[:], op=ALU.add)
                sbt = mid.tile([P, NB], BF16, tag="sbt")
                nc.vector.tensor_tensor(sbt[32:64, :], ck["S1b"][0:32, :],
                                        nt12[0:32, NB:2 * NB], op=ALU.subtract)
                zt1 = mid.tile([P, NB], BF16, tag="zt1t")
                nc.vector.tensor_tensor(zt1[0:32, :], rz1t[32:64, :], sbt[32:64, :], op=ALU.mult)
                nc.vector.tensor_tensor(ck["G1b"][0:32, :], nt12[0:32, NB:2 * NB], zt1[0:32, :], op=ALU.add)

                # ============ GLU1 ============
                U1 = psA.tile([P, 2 * NB], F32, tag="ps1024")
                nc.vector.memset(U1[32:128, NB:2 * NB][0:32, :], 0.0)
                nc.vector.memset(U1[64:128, NB:2 * NB], 0.0)
                emit(U1[:, 0:NB], mm_acc("glu1", 0, None))
                emit(U1[0:32, NB:2 * NB], mm_acc("glu1", 1, None))
                u1w = mid.tile([P, 2 * NB], BF16, tag="u1w")
                nc.scalar.activation(u1w[:], U1[:], AF.Sigmoid)
                nc.vector.tensor_tensor(ck["O1a"][:], ck["G1a"][:], u1w[:, 0:NB], op=ALU.mult)
                nc.vector.tensor_tensor(ck["O1b"][0:32, :], ck["G1b"][0:32, :],
                                        u1w[0:32, NB:2 * NB], op=ALU.mult)

                def gru_combine(Arz, NBw, s_chunk, g_out, width=P):
                    rz = mid.tile([width, 2 * NB], BF16, tag=f"rz{width}")
                    nc.scalar.activation(rz[:], Arz[:], AF.Sigmoid)
                    rh = mid.tile([width, NB], BF16, tag=f"rh{width}")
                    nc.vector.tensor_tensor(rh[:], rz[:, 0:NB], NBw[:, NB:2 * NB], op=ALU.mult)
                    npre = mid.tile([width, NB], BF16, tag=f"np{width}")
                    nc.vector.tensor_tensor(npre[:], NBw[:, 0:NB], rh[:], op=ALU.add)
                    nt = mid.tile([width, NB], BF16, tag=f"nt{width}")
                    nc.scalar.activation(nt[:], npre[:], AF.Tanh)
                    sub = mid.tile([width, NB], BF16, tag=f"sb{width}")
                    nc.vector.tensor_tensor(sub[:], s_chunk, nt[:], op=ALU.subtract)
                    zt = mid.tile([width, NB], BF16, tag=f"zt{width}")
                    nc.vector.tensor_tensor(zt[:], rz[:, NB:2 * NB], sub[:], op=ALU.mult)
                    nc.vector.tensor_tensor(g_out, nt[:], zt[:], op=ALU.add)

                # ============ GRU2 / GLU2 ============
                A2 = psA.tile([P, 2 * NB], F32, tag="ps1024")
                NB2 = psA.tile([P, 2 * NB], F32, tag="ps1024")
                emit(A2[:, 0:NB], mm_acc("gru2h", 0, None) + mm_acc("gru2i", 0, None))
                emit(A2[:, NB:2 * NB], mm_acc("gru2h", 1, None) + mm_acc("gru2i", 1, None))
                emit(NB2[:, NB:2 * NB], mm_acc("gru2h", 2, None))
                emit(NB2[:, 0:NB], mm_acc("gru2i", 2, None))
                gru_combine(A2, NB2, ck["S2"][:], ck["G2"][:], P)
                U2 = ps5.tile([P, NB], F32, tag="ps512")
                emit(U2[:], mm_acc("glu2", 0, None))
                u2 = mid.tile([P, NB], BF16, tag="u2")
                nc.scalar.activation(u2[:], U2[:], AF.Sigmoid)
                nc.vector.tensor_tensor(ck["O2"][:], ck["G2"][:], u2[:], op=ALU.mult)

                # ============ GRU3 / GLU3 ============
                A3 = psA.tile([P, 2 * NB], F32, tag="ps1024")
                NB3 = psA.tile([P, 2 * NB], F32, tag="ps1024")
                emit(A3[:, 0:NB], mm_acc("gru3h", 0, None) + mm_acc("gru3i", 0, None))
                emit(A3[:, NB:2 * NB], mm_acc("gru3h", 1, None) + mm_acc("gru3i", 1, None))
                emit(NB3[:, NB:2 * NB], mm_acc("gru3h", 2, None))
                emit(NB3[:, 0:NB], mm_acc("gru3i", 2, None))
                gru_combine(A3, NB3, ck["S3"][:], ck["G3"][:], P)
                U3 = ps5.tile([P, NB], F32, tag="ps512")
                emit(U3[:], mm_acc("glu3", 0, None))
                u3 = mid.tile([P, NB], BF16, tag="u3")
                nc.scalar.activation(u3[:], U3[:], AF.Sigmoid)
                nc.vector.tensor_tensor(ck["O3"][:], ck["G3"][:], u3[:], op=ALU.mult)

                # ============ skip / sig ============
                SK = ps5.tile([P, NB], F32, tag="ps512")
                emit(SK[:], mm_acc("skip", 0, None))
                nc.scalar.activation(ck["K0"][:], SK[:], AF.Tanh)
                UK = ps5.tile([P, NB], F32, tag="ps512")
                emit(UK[:], mm_acc("glusk", 0, None))
                uk = mid.tile([P, NB], BF16, tag="uk")
                nc.scalar.activation(uk[:], UK[:], AF.Sigmoid)
                nc.vector.tensor_tensor(ck["V0"][:], ck["K0"][:], uk[:], op=ALU.mult)
                SG = ps5.tile([P, NB], F32, tag="ps512")
                emit(SG[0:40, :], mm_acc("sig", 0, None))
                sigf = mid.tile([P, NB], F32, tag="sigf")
                nc.scalar.activation(sigf[0:40, :], SG[0:40, :], AF.Tanh)

                # sig-out: transpose back to batch-major, multiply by gain
                sigbm = gop.tile([P, RG, 40], F32, tag="sigbm")
                for rg in range(RG):
                    psT = ps5.tile([P, NB], F32, tag="ps512")
                    nc.tensor.transpose(psT[:, 0:40], sigf[0:40, rg * P:(rg + 1) * P],
                                        identF[0:40, 0:40])
                    nc.vector.tensor_scalar(sigbm[:, rg, :], psT[:, 0:40],
                                            gain2[:, rg, 0:1], None, op0=ALU.mult)
                nc.sync.dma_start(btview(o_sig, bt), sigbm[:])
                nc.sync.dma_start(btview(o_excn, bt, slice(216, 256)), sigbm[:])

                # ============ g1/g2/g3 outputs (bf16 xbar transpose + upcast) ====
                gbm = gop.tile([P, RG, 416], BF16, tag="gbm")
                nc.sync.dma_start_transpose(gbm[:, :, 0:128], ck["G1a"][:, :])
                nc.scalar.dma_start_transpose(gbm[:, :, 128:160], ck["G1b"][0:32, :])
                nc.sync.dma_start_transpose(gbm[:, :, 160:288], ck["G2"][:, :])
                nc.scalar.dma_start_transpose(gbm[:, :, 288:416], ck["G3"][:, :])
                gf = gop.tile([P, RG, 416], F32, tag="gf")
                nc.vector.tensor_copy(gf[:], gbm[:])
                nc.sync.dma_start(btview(o_g1, bt), gf[:, :, 0:160])
                nc.sync.dma_start(btview(o_g2, bt), gf[:, :, 160:288])
                nc.sync.dma_start(btview(o_g3, bt), gf[:, :, 288:416])

                # ============ s4_new / pp_new tails ============
                nc.sync.dma_start(btview(o_s4n, bt, slice(0, 80)), condf[:])
                nc.sync.dma_start(btview(o_s4n, bt, slice(80, 124)), predf[:])
                nc.sync.dma_start(btview(o_s4n, bt, slice(124, 164)), prevf[:])
                nc.sync.dma_start(btview(o_ppn, bt, slice(216, 256)), predf[:, :, 2:42])

            for step in range(n_bt + 1):
                if step < n_bt:
                    heads[step] = emit_headA(step)
                    emit_headB(step, heads[step])
                if step >= 1:
                    emit_body(step - 1, heads.pop(step - 1))

    nc.compile()
    return nc


# ---------------------------------------------------------------------------
# host entry
# ---------------------------------------------------------------------------
_NC_CACHE = {}


def _get_nc(n_bt=NBT):
    if n_bt not in _NC_CACHE:
        _NC_CACHE[n_bt] = build_nc(n_bt)
    return _NC_CACHE[n_bt]


def make_in_map(ins, lo, hi):
    """Build the per-core input map for rows [lo, hi)."""
    rows = hi - lo
    exc = np.ascontiguousarray(ins["exc_mem"][lo:hi]).astype(np.float32, copy=False)
    excf = np.zeros((EXC_PAD + rows * 256 + EXC_PAD, 1), np.float32)
    excf[EXC_PAD:EXC_PAD + rows * 256, 0] = exc.ravel()
    pidx = np.zeros((rows, 2), np.int32)
    pidx[:, 0] = np.asarray(ins["period"][lo:hi]).astype(np.int32)
    pidx[:, 1] = EXC_PAD + np.arange(rows, dtype=np.int32) * 256
    iota2 = np.broadcast_to(np.arange(44, dtype=np.float32) - 2.0, (P, 4, 44))
    blob = build_blob(ins)
    return {
        "cond_d": np.ascontiguousarray(ins["cond"][lo:hi], np.float32),
        "s1_d": np.ascontiguousarray(ins["s1"][lo:hi], np.float32),
        "s2_d": np.ascontiguousarray(ins["s2"][lo:hi], np.float32),
        "s3_d": np.ascontiguousarray(ins["s3"][lo:hi], np.float32),
        "s4_d": np.ascontiguousarray(ins["s4"][lo:hi], np.float32),
        "exc2d_d": exc,
        "excf_d": excf,
        "ppin_d": np.ascontiguousarray(ins["prev_pred"][lo:hi, 40:256], np.float32),
        "pidx_d": pidx,
        "blob_d": blob,
        "wcg_d": np.broadcast_to(np.asarray(ins["W_cg"], np.float32).reshape(1, 80),
                                 (P, 80)).copy(),
        "iota2_d": np.ascontiguousarray(iota2.reshape(P, 176)),
        "bcg_d": np.full((P, 1), float(np.asarray(ins["b_cg"]).ravel()[0]), np.float32),
        "bpg_d": np.asarray(ins["b_pg"], np.float32).reshape(4, 1),
    }


def kernel(**inputs):
    from concourse.bass_utils import run_bass_kernel_spmd
    nc = _get_nc()
    in_maps = [make_in_map(inputs, c * R, (c + 1) * R) for c in range(N_CORES)]
    res = run_bass_kernel_spmd(nc, in_maps, core_ids=list(range(N_CORES)),
                               trace=bool(int(os.environ.get("K_TRACE", "0"))))
    outs = res.results
    if bool(int(os.environ.get("K_TRACE", "0"))):
        kernel.last_result = res
    cat = lambda k: np.concatenate([outs[c][k] for c in range(N_CORES)], 0)
    return (cat("sig_o"), cat("excn_o"), cat("ppn_o"),
            cat("g1_o"), cat("g2_o"), cat("g3_o"), cat("s4n_o"))
